# revision 32
# baseline (speedup 1.0000x reference)
"""Causal self-attention (B=2, T=2048, E=2048, H=16, D=128) on 8 NeuronCores.

Sharding: tensor-parallel over heads — each core owns 2 heads (256 features).
Per core: QKV projections for its head slice, RoPE, causal attention in S^T
layout (keys on partitions), and a partial output projection against its Wo
row-slice. The host sums the 8 bf16 partials and adds bo.

v2: all matmul operands bf16 (full PE rate, half the DMA/SBUF of fp32r),
pair-batched Exp from 2-bank PSUM tiles, diagonal-pair trimming, and
O-projection work interleaved into the next chunk's projection phase to
keep the PE dense.

v6 (344us/rep, from 475us): softmax denominators accumulated chunk-level on
DVE with a single broadcasting ones-matmul per (chunk, head) instead of
per-pair; reciprocal via reciprocal_approx_fast (5x cheaper than the exact
iterative divide); RoPE rotation via SBUF->SBUF partition-swap DMA (sign
folded into the sin tables) instead of a PE matmul; output-projection PSUM
drains split between Act and DVE so neither engine's FIFO stalls the
bias-activation chain; per-head softmax tails interleaved into the last PV
pair so the PSUM pool recycles before the next chunk's projections.
(fp8/DoubleRow was evaluated and rejected: e4m3 on any matmul stage pushes
rel err to 2.0-3.5e-2, over the 2e-2 gate.)
"""
import sys

sys.path.insert(0, "/opt/trn_rl_repo")

import numpy as np
import ml_dtypes

import concourse.mybir as mybir
import concourse.tile as tile
from concourse import bacc
from concourse.bass_utils import run_bass_kernel_spmd

B, T, E, H = 2, 2048, 2048, 16
D = E // H            # 128 head dim
N_CORES = 8
HPC = H // N_CORES    # 2 heads per core
FPC = HPC * D         # 256 features per core
ROPE_BASE = 10000.0

CH = 512              # t-chunk (moving free dim)
NCH = T // CH         # 4 chunks
KT = E // 128         # 16 contraction tiles
NTT = T // 128        # 16 t-subtiles

f32 = mybir.dt.float32
bf16 = mybir.dt.bfloat16
BF = ml_dtypes.bfloat16

PHASE_MARKS = []


def build_nc(reps: int = 1):
    """Build the per-core Bass program. reps>1 wraps the body in a hardware
    repeat loop (identical work each iteration) for slope-timing."""
    nc = bacc.Bacc("TRN2", target_bir_lowering=False, debug=False,
                   num_devices=N_CORES)

    def mark(label):
        PHASE_MARKS.append((label, int(nc.get_next_instruction_name()[2:])))

    xprep = nc.dram_tensor("xprep", [B, NCH, 128, KT, CH], bf16,
                           kind="ExternalInput")
    wqkv = nc.dram_tensor("wqkv", [128, KT, 3 * FPC], bf16,
                          kind="ExternalInput")
    wo = nc.dram_tensor("wo", [128, HPC, E], bf16, kind="ExternalInput")
    tabs = nc.dram_tensor("tabs", [128, 4, T], bf16, kind="ExternalInput")
    masks = nc.dram_tensor("masks", [128, 4, CH], bf16, kind="ExternalInput")
    bqkd = nc.dram_tensor("bqkd", [128, 2 * HPC], f32, kind="ExternalInput")
    bvbd = nc.dram_tensor("bvbd", [128, FPC], bf16, kind="ExternalInput")
    ones128 = nc.dram_tensor("ones128", [128, 128], bf16, kind="ExternalInput")
    y = nc.dram_tensor("y", [B, NTT, NCH, 128, CH], bf16,
                       kind="ExternalOutput")

    Exp = mybir.ActivationFunctionType.Exp
    Identity = mybir.ActivationFunctionType.Identity

    with tile.TileContext(nc) as tc:
        with (
            nc.allow_low_precision(reason="bf16 kernel; tolerance 2e-2"),
            tc.tile_pool(name="const", bufs=1) as constp,
            tc.tile_pool(name="xc", bufs=2) as xcp,
            tc.tile_pool(name="kv", bufs=1) as kvp,
            tc.tile_pool(name="qt", bufs=2) as qtp,
            tc.tile_pool(name="rope", bufs=2) as ropep,
            tc.tile_pool(name="pt", bufs=4) as ptp,
            tc.tile_pool(name="ot", bufs=2) as otp,
            tc.tile_pool(name="yb", bufs=4) as ybp,
            tc.tile_pool(name="small", bufs=2) as smallp,
            tc.tile_pool(name="pvps", bufs=2, space="PSUM") as pvp,
            tc.tile_pool(name="sps", bufs=2, space="PSUM") as sp_pool,
            tc.tile_pool(name="oyps", bufs=2, space="PSUM") as oyp,
        ):
            # ---- resident constants (loaded once per exec) ----
            w_sb = constp.tile([128, KT, 3 * FPC], bf16, tag="w")
            nc.sync.dma_start(out=w_sb[:, :, :], in_=wqkv.ap())
            wo_sb = constp.tile([128, HPC, E], bf16, tag="wo")
            nc.sync.dma_start(out=wo_sb[:, :, :], in_=wo.ap())
            tab_sb = constp.tile([128, 4, T], bf16, tag="tabs")
            nc.sync.dma_start(out=tab_sb[:, :, :], in_=tabs.ap())
            mask_sb = constp.tile([128, 4, CH], bf16, tag="masks")
            nc.sync.dma_start(out=mask_sb[:, :, :], in_=masks.ap())
            bqk_sb = constp.tile([128, 2 * HPC], f32, tag="bqk")
            nc.sync.dma_start(out=bqk_sb[:, :], in_=bqkd.ap())
            bvb_sb = constp.tile([128, FPC], bf16, tag="bvb")
            nc.sync.dma_start(out=bvb_sb[:, :], in_=bvbd.ap())
            ones_sb = constp.tile([128, 128], bf16, tag="ones")
            nc.sync.dma_start(out=ones_sb[:, :], in_=ones128.ap())

            def body():
                bwork = []  # pending O-projection emit thunks

                def drain_b(n):
                    for _ in range(min(n, len(bwork))):
                        bwork.pop(0)()

                def load_chunk(b, c):
                    xh = xcp.tile([128, KT, CH], bf16, tag="xc",
                                  name="xch")
                    nc.sync.dma_start(out=xh[:, :, :],
                                      in_=xprep.ap()[b][c])
                    return xh

                nxt = load_chunk(0, 0)
                for b in range(B):
                    kt_sb = [kvp.tile([128, T], bf16, tag=f"kt{h}",
                                      name=f"ktt{h}") for h in range(HPC)]
                    v_sb = kvp.tile([128, NTT, FPC], bf16, tag="v")

                    for c in range(NCH):
                        xh = nxt
                        # prefetch the next chunk's x as early as possible
                        if c + 1 < NCH:
                            nxt = load_chunk(b, c + 1)
                        elif b + 1 < B:
                            nxt = load_chunk(b + 1, 0)
                        cc = slice(c * CH, (c + 1) * CH)

                        # ---- stage A: q/k projections + rope ----
                        mark(f"b{b}c{c}:Ax")
                        qt_sb = [qtp.tile([128, CH], bf16, tag=f"qt{h}",
                                          name=f"qtt{h}") for h in range(HPC)]
                        for which in (1, 0):        # k first, then q
                            for h in range(HPC):
                                ps = pvp.tile([128, CH], f32, tag="pv",
                                              name="projps")
                                fofs = which * FPC + h * D
                                for k in range(KT):
                                    # col-tiled halves: 64-col LDWs pipeline
                                    # behind the concurrent half-matmuls
                                    nc.tensor.matmul(
                                        ps[0:64, :],
                                        w_sb[:, k, fofs:fofs + 64],
                                        xh[:, k, :],
                                        start=(k == 0), stop=(k == KT - 1),
                                        tile_position=(0, 0),
                                        skip_group_check=True)
                                    nc.tensor.matmul(
                                        ps[64:128, :],
                                        w_sb[:, k, fofs + 64:fofs + D],
                                        xh[:, k, :],
                                        start=(k == 0), stop=(k == KT - 1),
                                        tile_position=(0, 64),
                                        skip_group_check=True)
                                col = which * HPC + h
                                qb = ropep.tile([128, CH], bf16, tag="qb")
                                nc.scalar.activation(
                                    qb[:, :], ps[:, :], Identity,
                                    bias=bqk_sb[:, col:col + 1])
                                drain_b(2)  # cover the qb latency on PE
                                # partition-rotated copy (d <-> d+64) via
                                # SBUF->SBUF DMA; sign folded into sin table
                                qsw = ropep.tile([128, CH], bf16, tag="qsw")
                                nc.gpsimd.dma_start(out=qsw[0:64, :],
                                                    in_=qb[64:128, :])
                                nc.gpsimd.dma_start(out=qsw[64:128, :],
                                                    in_=qb[0:64, :])
                                ct = tab_sb[:, 2 * which, cc]
                                st = tab_sb[:, 2 * which + 1, cc]
                                t1 = ropep.tile([128, CH], bf16, tag="t1")
                                nc.vector.tensor_mul(t1[:, :], qb[:, :], ct)
                                t2 = ropep.tile([128, CH], bf16, tag="t2")
                                nc.vector.tensor_mul(t2[:, :], qsw[:, :], st)
                                if which == 0:
                                    nc.vector.tensor_add(qt_sb[h][:, :],
                                                         t1[:, :], t2[:, :])
                                else:
                                    nc.vector.tensor_add(kt_sb[h][:, cc],
                                                         t1[:, :], t2[:, :])
                                drain_b(2)

                        # ---- attention pair machinery (used across stages) ----
                        njt = 4 * (c + 1)
                        npairs = njt // 2
                        pts = {}
                        acc = {h: ptp.tile([128, CH], bf16, tag=f"acc{h}",
                                           name=f"acct{h}", bufs=2)
                               for h in range(HPC)}

                        def emit_pair(h, p):
                            # per-subtile causal trim: key subtile j only
                            # matters for queries >= 128*(j-4c)
                            jd0 = 2 * p - 4 * c
                            jd1 = jd0 + 1
                            lo0 = 128 * max(0, jd0)
                            lo1 = 128 * max(0, jd1)
                            s2 = sp_pool.tile([128, 2, CH], f32,
                                              tag="s2", name="s2")
                            for jj, loj in ((0, lo0), (1, lo1)):
                                j = 2 * p + jj
                                nc.tensor.matmul(
                                    s2[0:64, jj, loj:],
                                    kt_sb[h][:, j * 128:j * 128 + 64],
                                    qt_sb[h][:, loj:],
                                    start=True, stop=True,
                                    tile_position=(0, 0),
                                    skip_group_check=True)
                                nc.tensor.matmul(
                                    s2[64:128, jj, loj:],
                                    kt_sb[h][:, j * 128 + 64:j * 128 + 128],
                                    qt_sb[h][:, loj:],
                                    start=True, stop=True,
                                    tile_position=(0, 64),
                                    skip_group_check=True)
                            pt2 = ptp.tile([128, 2, CH], bf16,
                                           tag="pt", name="pt2")
                            # Exp only over PSUM that the S matmuls wrote:
                            # reading never-written PSUM can raise a parity
                            # error and kill the exec unit
                            if lo0 == lo1:
                                nc.scalar.activation(pt2[:, :, lo0:],
                                                     s2[:, :, lo0:], Exp)
                            else:
                                nc.scalar.activation(pt2[:, 0, lo0:],
                                                     s2[:, 0, lo0:], Exp)
                                nc.scalar.activation(pt2[:, 1, lo1:],
                                                     s2[:, 1, lo1:], Exp)
                            # triangle mask only on the diagonal 128-block
                            tri = mask_sb[:, 0, 0:128]
                            for jj, loj, jd in ((0, lo0, jd0), (1, lo1, jd1)):
                                if jd >= 0:
                                    nc.vector.tensor_mul(
                                        pt2[:, jj, loj:loj + 128],
                                        pt2[:, jj, loj:loj + 128],
                                        tri)
                            # accumulate softmax denominators into acc[h]
                            # (chunk-level; one ones-matmul per (c,h) later)
                            if p == 0:
                                if lo1 == 0:
                                    nc.vector.tensor_add(acc[h][:, :],
                                                         pt2[:, 0, :],
                                                         pt2[:, 1, :])
                                else:  # c == 0: first pair is diagonal
                                    nc.vector.tensor_copy(acc[h][:, 0:lo1],
                                                          pt2[:, 0, 0:lo1])
                                    nc.vector.tensor_add(acc[h][:, lo1:],
                                                         pt2[:, 0, lo1:],
                                                         pt2[:, 1, lo1:])
                            elif lo0 == lo1:
                                psum_t = ptp.tile([128, CH], bf16,
                                                  tag="psum_t", name="psum_t",
                                                  bufs=3)
                                nc.vector.tensor_add(psum_t[:, lo0:],
                                                     pt2[:, 0, lo0:],
                                                     pt2[:, 1, lo0:])
                                nc.vector.tensor_add(acc[h][:, lo0:],
                                                     acc[h][:, lo0:],
                                                     psum_t[:, lo0:])
                            else:  # diagonal pair: disjoint valid ranges
                                nc.vector.tensor_add(acc[h][:, lo0:],
                                                     acc[h][:, lo0:],
                                                     pt2[:, 0, lo0:])
                                nc.vector.tensor_add(acc[h][:, lo1:],
                                                     acc[h][:, lo1:],
                                                     pt2[:, 1, lo1:])
                            pts[(h, p)] = (pt2, lo0, lo1)

                        # ---- stage A2: v projection ----
                        mark(f"b{b}c{c}:Av")
                        # allocate PV accumulators early: the pool wait on the
                        # previous chunk's drained yp tiles resolves during Av
                        o_ps = [oyp.tile([128, CH], f32, tag="oy",
                                         name=f"ops{h}") for h in range(HPC)]
                        for tsub in range(4):
                            vps = pvp.tile([128, FPC], f32, tag="pv",
                                           name="vps")
                            for k in range(KT):
                                nc.tensor.matmul(
                                    vps[0:64, :],
                                    xh[:, k, tsub * 128:tsub * 128 + 64],
                                    w_sb[:, k, 2 * FPC:3 * FPC],
                                    start=(k == 0), stop=(k == KT - 1),
                                    tile_position=(0, 0),
                                    skip_group_check=True)
                                nc.tensor.matmul(
                                    vps[64:128, :],
                                    xh[:, k, tsub * 128 + 64:tsub * 128 + 128],
                                    w_sb[:, k, 2 * FPC:3 * FPC],
                                    start=(k == 0), stop=(k == KT - 1),
                                    tile_position=(0, 64),
                                    skip_group_check=True)
                            nc.vector.tensor_add(v_sb[:, c * 4 + tsub, :],
                                                 vps[:, :], bvb_sb[:, :])
                            drain_b(2)

                        drain_b(len(bwork))

                        # ---- stage S: attention, heads interleaved,
                        #      softmax denominators accumulated per quad ----
                        mark(f"b{b}c{c}:S")
                        for h in range(HPC):
                            emit_pair(h, 0)
                        ot_c = [None] * HPC

                        def emit_tail(h):
                            # denominator broadcast + normalize, as soon as
                            # this head's last pair lands (frees pvp early)
                            rsp = pvp.tile([128, CH], f32, tag="pv",
                                           name=f"rsps{h}")
                            nc.tensor.matmul(
                                rsp[0:64, :],
                                ones_sb[:, 0:64],
                                acc[h][:, :],
                                start=True, stop=True,
                                tile_position=(0, 0),
                                skip_group_check=True)
                            nc.tensor.matmul(
                                rsp[64:128, :],
                                ones_sb[:, 64:128],
                                acc[h][:, :],
                                start=True, stop=True,
                                tile_position=(0, 64),
                                skip_group_check=True)
                            rcp = smallp.tile([128, CH], f32, tag="rcp")
                            nc.vector.reciprocal_approx_fast(
                                rcp[:, :], rsp[:, :])
                            ot = otp.tile([128, CH], bf16, tag=f"ot{h}",
                                          name=f"ott{h}")
                            nc.vector.tensor_mul(ot[:, :], o_ps[h][:, :],
                                                 rcp[:, :])
                            ot_c[h] = ot

                        for p in range(npairs):
                            for h in range(HPC):
                                if p + 1 < npairs:
                                    emit_pair(h, p + 1)
                                pt2, lo0, lo1 = pts.pop((h, p))
                                for jj, loj in ((0, lo0), (1, lo1)):
                                    j = 2 * p + jj
                                    nc.tensor.matmul(
                                        o_ps[h][0:64, loj:],
                                        v_sb[:, j, h * D:h * D + 64],
                                        pt2[:, jj, loj:],
                                        start=(p == 0 and jj == 0),
                                        stop=(p == npairs - 1 and jj == 1),
                                        tile_position=(0, 0),
                                        skip_group_check=True)
                                    nc.tensor.matmul(
                                        o_ps[h][64:128, loj:],
                                        v_sb[:, j, h * D + 64:h * D + D],
                                        pt2[:, jj, loj:],
                                        start=(p == 0 and jj == 0),
                                        stop=(p == npairs - 1 and jj == 1),
                                        tile_position=(0, 64),
                                        skip_group_check=True)
                                if p == npairs - 1:
                                    emit_tail(h)

                        # ---- stage B: output projection (deferred) ----
                        mark(f"b{b}c{c}:B")
                        ots = ot_c

                        def mk_yp(ti, tloc, gc, ots=ots, b=b):
                            # first half drains (popped while k-projections
                            # run) on Act; later ones (near q/v) on DVE so the
                            # Act FIFO never delays the qb bias-drains
                            on_act = tloc < 2

                            def emit():
                                yp = oyp.tile([128, CH], f32, tag="oy",
                                              name="yp")
                                for h in range(HPC):
                                    nc.tensor.matmul(
                                        yp[0:64, :],
                                        ots[h][:, tloc * 128:tloc * 128 + 64],
                                        wo_sb[:, h, gc * CH:(gc + 1) * CH],
                                        start=(h == 0), stop=(h == HPC - 1),
                                        tile_position=(0, 0),
                                        skip_group_check=True)
                                    nc.tensor.matmul(
                                        yp[64:128, :],
                                        ots[h][:, tloc * 128 + 64:
                                               tloc * 128 + 128],
                                        wo_sb[:, h, gc * CH:(gc + 1) * CH],
                                        start=(h == 0), stop=(h == HPC - 1),
                                        tile_position=(0, 64),
                                        skip_group_check=True)
                                yb = ybp.tile([128, CH], bf16, tag="yb")
                                if on_act:
                                    nc.scalar.activation(yb[:, :], yp[:, :],
                                                         Identity)
                                else:
                                    nc.vector.tensor_copy(yb[:, :], yp[:, :])
                                nc.sync.dma_start(out=y.ap()[b][ti][gc],
                                                  in_=yb[:, :])
                            return emit

                        for tloc in range(4):
                            for gc in range(NCH):
                                bwork.append(mk_yp(4 * c + tloc, tloc, gc))
                # tail: flush remaining O-projection work
                drain_b(len(bwork))
                mark("end")

            if reps == 1:
                body()
            else:
                with tc.For_i(0, reps, 1):
                    body()

    nc.compile()
    return nc


def host_inputs(x, Wq, bq, Wk, bk, Wv, bv, Wo, bo):
    """Prepare per-core input maps from the full problem inputs."""
    x = np.asarray(x, np.float32)
    # x tile-major: [B, NCH, 128(e-sub), KT, CH]
    xp = np.ascontiguousarray(
        x.reshape(B, NCH, CH, KT, 128).transpose(0, 1, 4, 3, 2)
    ).astype(BF)

    # RoPE tables, 1-indexed positions, 1/sqrt(D) folded into the Q tables
    j = np.arange(D // 2, dtype=np.float64)
    thetas = ROPE_BASE ** (-2.0 * j / D)
    m = np.arange(1, T + 1, dtype=np.float64)
    ang = m[:, None] * thetas[None, :]          # [T, D/2]
    ang = np.concatenate([ang, ang], axis=1)    # [T, D]
    s = 1.0 / np.sqrt(D)
    # sign of the rotated term folded into the sin tables: the kernel builds
    # qsw[d] = q[(d+64)%128], and rot(q)[d] = -q[d+64] (d<64), q[d-64] (d>=64)
    sgn = np.where(np.arange(D) < D // 2, -1.0, 1.0)[:, None]  # [128,1]
    tabs = np.stack([
        (np.cos(ang) * s).T, (np.sin(ang) * s).T * sgn,
        np.cos(ang).T, np.sin(ang).T * sgn,
    ], axis=1).astype(BF)                        # [128, 4, T]

    # causal masks: mask_p[kk, qq] = qq >= 128p + kk
    kk = np.arange(128)[:, None]
    qq = np.arange(CH)[None, :]
    masks = np.stack([(qq >= 128 * p + kk) for p in range(4)],
                     axis=1).astype(BF)          # [128, 4, CH]

    ones128 = np.ones((128, 128), np.float32).astype(BF)

    in_maps = []
    for cr in range(N_CORES):
        fs = slice(cr * FPC, (cr + 1) * FPC)
        wcat = np.concatenate([Wq[fs].T, Wk[fs].T, Wv[fs].T], axis=1)
        wqkv = np.ascontiguousarray(
            wcat.reshape(KT, 128, 3 * FPC).transpose(1, 0, 2)
        ).astype(BF)                               # [128, KT, 768]
        woT = np.asarray(Wo[:, fs].T, np.float32)  # [FPC, E]
        wop = np.ascontiguousarray(
            woT.reshape(HPC, 128, E).transpose(1, 0, 2)
        ).astype(BF)                               # [128, HPC, E]
        bqk_cols = np.stack([
            bq[fs][:D], bq[fs][D:], bk[fs][:D], bk[fs][D:],
        ], axis=1).astype(np.float32)              # [128, 4]
        bvb = np.broadcast_to(
            np.asarray(bv[fs], np.float32)[None, :], (128, FPC)
        ).astype(BF)                               # [128, 256]
        in_maps.append({
            "xprep": xp,
            "wqkv": wqkv,
            "wo": wop,
            "tabs": tabs,
            "masks": masks,
            "bqkd": bqk_cols,
            "bvbd": np.ascontiguousarray(bvb),
            "ones128": ones128,
        })
    return in_maps


_NC_CACHE = {}


def get_nc(reps: int = 1):
    if reps not in _NC_CACHE:
        _NC_CACHE[reps] = build_nc(reps)
    return _NC_CACHE[reps]


def kernel(x, Wq, bq, Wk, bk, Wv, bv, Wo, bo):
    in_maps = host_inputs(x, Wq, bq, Wk, bk, Wv, bv, Wo, bo)
    nc = get_nc(1)
    res = run_bass_kernel_spmd(nc, in_maps, list(range(N_CORES)))
    out = np.zeros((B, T, E), np.float64)
    for cr in range(N_CORES):
        yp = res.results[cr]["y"].astype(np.float32)  # [B, NTT, NCH, 128, CH]
        out += yp.transpose(0, 1, 3, 2, 4).reshape(B, T, E).astype(np.float64)
    out += np.asarray(bo, np.float64)[None, None, :]
    return out.astype(np.float32)

